# revision 16
# baseline (speedup 1.0000x reference)
"""AttnConv2d Trainium2 kernel (8-core SPMD, data-parallel over batch).

Problem (per sample b):
  CK  = conv3x3_same(x1, key_w);  CQ1 = conv3x3_same(x1, q1_w);  CQ2 = conv3x3_same(x2, q2_w)
  For each kernel position p (3x3 grid phase) the conv outputs are unfolded with
  stride 3 -> per-p pixel sets of L=1024 pixels.
  A1[o,c,p] = sum_l CQ1[o,pix] * CK[c,pix];  A2 likewise with CQ2.
  attn = (A1+pos1)*(A2+pos2);  global mean/std normalize (over the whole batch);
  out  = conv3x3_same(x1, attn reshaped to [Cout,Cin,3,3] per sample).

Device strategy (one sample per NeuronCore):
  - All matmuls in float32r (fp32 storage, ~1e-4 matmul rel err, 4x fp32 speed).
  - Convs computed in transposed layout [pixel, channel] (lhsT = shifted image
    band, rhs = weights), tiled so each 128-pixel tile belongs to one kernel
    position p; A accumulated as A^T[c, p*256+o] via PSUM + DVE adds.
  - Normalization is deferred via linearity: the device returns the
    UN-normalized U = conv(x1, attn_raw) plus per-core attn sum/sumsq partials.
    Host computes global a,b and finishes out = a*U + b*boxsum(x1) (boxsum is
    conv(x1, ones) = the contribution of the constant shift b to the conv).
"""

import os
import sys
from contextlib import ExitStack

import numpy as np

for _p in ("/opt/trn_rl_repo",):
    if _p not in sys.path and os.path.isdir(_p):
        sys.path.append(_p)

import concourse.bacc as bacc
import concourse.tile as tile
import concourse.mybir as mybir
from concourse.bass_utils import run_bass_kernel_spmd

F32 = mybir.dt.float32
F32R = mybir.dt.float32r

B, CIN, COUT, H, W = 8, 256, 256, 96, 96
K = 3
K2 = K * K
HW = H * W          # 9216
NORM_SCALE = 1.0
N_CORES = 8

_CACHE = {}


def build_nc():
    nc = bacc.Bacc("TRN2", target_bir_lowering=False, debug=False,
                   enable_asserts=True, num_devices=N_CORES)
    x1 = nc.dram_tensor("x1", [CIN, H, W], F32R, kind="ExternalInput")
    x2 = nc.dram_tensor("x2", [CIN, H, W], F32R, kind="ExternalInput")
    wkq1 = nc.dram_tensor("wkq1", [CIN, K2 * 2 * COUT], F32R, kind="ExternalInput")
    wq2 = nc.dram_tensor("wq2", [CIN, K2 * COUT], F32R, kind="ExternalInput")
    pos1t = nc.dram_tensor("pos1t", [CIN, K2 * COUT], F32, kind="ExternalInput")
    pos2t = nc.dram_tensor("pos2t", [CIN, K2 * COUT], F32, kind="ExternalInput")
    u_out = nc.dram_tensor("u", [COUT, HW], F32, kind="ExternalOutput")
    stats_out = nc.dram_tensor("stats", [128, 2], F32, kind="ExternalOutput")

    with tile.TileContext(nc) as tc:
        _emit(nc, tc, x1, x2, wkq1, wq2, pos1t, pos2t, u_out, stats_out)
    nc.compile()
    return nc


def _strided_view(ap, free_dims, offset_elems):
    """Custom strided free-dim view of an SBUF AP (keeps partition dim)."""
    v = ap.copy()
    a = v.ap
    while len(a) > 1:
        a.pop()
    for d in free_dims:
        a.append(list(d))
    v.ap = a
    v.offset = ap.offset + offset_elems
    return v


# lhsT block-start row in the per-phase band for tap offset v = dy+ty-1:
#   dy'=v%3 phase blocks: dy'=0 at rows 0..4 (ly' 4m..4m+4), dy'=1 at rows
#   5..8 (ly' 4m..4m+3), dy'=2 at rows 9..13 (ly' 4m-1..4m+3)
_SROW = {-1: 9, 0: 0, 1: 5, 2: 10, 3: 1}
# per-phase (dyp -> (block base, n rows, staging row of local r=0, ...)):
#   staging row of phase-local row r:  dy'=0: 3r+1, dy'=1: 3r+2, dy'=2: 3r
_PHASE_ROWS = {0: (0, 5, 1), 1: (5, 4, 2), 2: (9, 5, 0)}


def _phase_b(nc, tc, ctx, x1, x2, wkq1, wq2, a1t, a2t, pos1t, pos2t):
    """Convs in [pixel, channel] layout + A^T accumulation.

    Rowwise staging band (zero-padded 98-wide) is DMA'd from DRAM, then
    gpsimd copies split it into per-phase grids:
      band[c, j, dxx, drow, lx] = x[j*128+c, y(drow), 3*lx + (dxx-1)]
    so a tap (p=(dy,dx), t=(ty,tx)) reads the CONTIGUOUS 128-px block
    band[:, j, dx+tx, SROW[dy+ty-1] : +4, :]  (4 subgrid rows x 32).
    """
    wpool = ctx.enter_context(tc.tile_pool(name="weights", bufs=1))
    stpool = ctx.enter_context(tc.tile_pool(name="staging", bufs=1))
    bpool = ctx.enter_context(tc.tile_pool(name="bands", bufs=1))
    cpool = ctx.enter_context(tc.tile_pool(name="convsb", bufs=2))
    convps = ctx.enter_context(tc.tile_pool(name="convps", bufs=2, space="PSUM"))
    aps = ctx.enter_context(tc.tile_pool(name="aps", bufs=2, space="PSUM"))

    SR = 14
    st1 = stpool.tile([128, 2, SR, 98], F32R, tag="st1")
    st2 = stpool.tile([128, 2, SR, 98], F32R, tag="st2")
    nc.gpsimd.memset(st1[:].bitcast(F32), 0.0)
    nc.gpsimd.memset(st2[:].bitcast(F32), 0.0)
    # m=0 staging prefetch BEFORE the weight DMAs so the first split copies
    # (startup critical path) are not stuck behind ~7MB of weights in-queue
    for stg, xdram in ((st1, x1), (st2, x2)):
        for j in range(2):
            nc.sync.dma_start(stg[:, j, 1:SR, 1:97],
                              xdram[j * 128:(j + 1) * 128, 0:SR - 1, :])

    # wkq1: [cin, tau, 512] = [wk_tau | wq1_tau] packed on host.
    # Chunked per (j, tap) so the first conv matmuls only wait on one piece.
    wkq1_t = wpool.tile([128, 2, K2 * 2 * COUT], F32R, tag="wkq1")
    wq2_t = wpool.tile([128, 2, K2 * COUT], F32R, tag="wq2")
    for t in range(K2):
        for j in range(2):
            nc.sync.dma_start(wkq1_t[:, j, t * 512:(t + 1) * 512],
                              wkq1[j * 128:(j + 1) * 128, t * 512:(t + 1) * 512])
            nc.sync.dma_start(wq2_t[:, j, t * COUT:(t + 1) * COUT],
                              wq2[j * 128:(j + 1) * 128, t * COUT:(t + 1) * COUT])
    x1b = [bpool.tile([128, 2, 5, SR, 32], F32R, tag=f"x1b{i}", name=f"x1b{i}") for i in range(2)]
    x2b = [bpool.tile([128, 2, 5, SR, 32], F32R, tag=f"x2b{i}", name=f"x2b{i}") for i in range(2)]

    for m in range(8):
        bx1, bx2 = x1b[m % 2], x2b[m % 2]
        y0 = 12 * m - 1                       # img row of staging row 0
        r0, r1 = max(0, -y0), min(SR, 96 - y0)
        if m == 7:
            # m=6 filled staging row 13 with real data; y=96 must be zero
            nc.gpsimd.memset(st1[:, :, SR - 1, :].bitcast(F32), 0.0)
            nc.gpsimd.memset(st2[:, :, SR - 1, :].bitcast(F32), 0.0)
        if m > 0:
            for stg, xdram in ((st1, x1), (st2, x2)):
                for j in range(2):
                    nc.sync.dma_start(stg[:, j, r0:r1, 1:97],
                                      xdram[j * 128:(j + 1) * 128, y0 + r0:y0 + r1, :])
        else:
            # staging for m=0 was prefetched before the weights; initialize
            # accumulators with pos (A_final = A_raw + pos for free) on the
            # gpsimd SWDGE queues to stay out of the startup-critical path
            for j in range(2):
                nc.gpsimd.dma_start(a1t[:, j, :], pos1t[j * 128:(j + 1) * 128, :])
                nc.gpsimd.dma_start(a2t[:, j, :], pos2t[j * 128:(j + 1) * 128, :])
        # phase-split copies: staging -> band, spread across idle engines
        # band[c, j, dxx, base+r, lx] = staging[c, j, srow0+3r, 3*lx + dxx]
        idx = 0
        for stg, band in ((st1, bx1), (st2, bx2)):
            for j in range(2):
                sv = stg[:, j, :, :]           # [128, 14, 98]
                for dyp in range(3):
                    base, nr, srow0 = _PHASE_ROWS[dyp]
                    dst = band[:, j, :, base:base + nr, :]         # [128, 5, nr, 32]
                    src = _strided_view(sv, [[1, 5], [3 * 98, nr], [3, 32]],
                                        srow0 * 98)
                    eng = (nc.scalar, nc.scalar, nc.gpsimd, nc.vector,
                           nc.scalar, nc.scalar, nc.gpsimd, nc.vector,
                           nc.scalar, nc.scalar, nc.gpsimd, nc.scalar)[idx]
                    if eng is nc.scalar:
                        nc.scalar.copy(dst, src)
                    else:
                        eng.tensor_copy(dst, src)
                    idx += 1
        for p in range(K2):
            dy, dx = p // 3, p % 3
            ckq1_ps = convps.tile([128, 2 * COUT], F32, tag="ckq1_ps")
            q2_ps = convps.tile([128, COUT], F32, tag="q2_ps")
            for j in range(2):
                for t in range(K2):
                    ty, tx = t // 3, t % 3
                    v = dy + ty - 1
                    s = _SROW[v]
                    dxx = dx + tx            # (dx+tx-1) + 1
                    lhs1 = bx1[:, j, dxx, s:s + 4, :]
                    lhs2 = bx2[:, j, dxx, s:s + 4, :]
                    st = (j == 0 and t == 0)
                    sp = (j == 1 and t == K2 - 1)
                    nc.tensor.matmul(ckq1_ps[:], lhs1,
                                     wkq1_t[:, j, t * 512:(t + 1) * 512], start=st, stop=sp)
                    nc.tensor.matmul(q2_ps[:], lhs2,
                                     wq2_t[:, j, t * COUT:(t + 1) * COUT], start=st, stop=sp)
            ckt = cpool.tile([128, COUT], F32R, tag="ckt")
            q12t = cpool.tile([128, 2 * COUT], F32R, tag="q12t")
            nc.vector.tensor_copy(ckt[:], ckq1_ps[:, 0:COUT])
            nc.vector.tensor_copy(q12t[:, 0:COUT], ckq1_ps[:, COUT:2 * COUT])
            nc.vector.tensor_copy(q12t[:, COUT:2 * COUT], q2_ps[:])

            a12_ps = aps.tile([128, 2, 512], F32, tag="a12_ps")
            for j in range(2):
                nc.tensor.matmul(a12_ps[:, j, :], ckt[:, j * 128:(j + 1) * 128], q12t[:],
                                 start=True, stop=True)
            psl = slice(p * COUT, (p + 1) * COUT)
            nc.vector.tensor_tensor(out=a1t[:, :, psl], in0=a1t[:, :, psl],
                                    in1=a12_ps[:, :, 0:COUT], op=mybir.AluOpType.add)
            nc.vector.tensor_tensor(out=a2t[:, :, psl], in0=a2t[:, :, psl],
                                    in1=a12_ps[:, :, COUT:2 * COUT], op=mybir.AluOpType.add)


def _phase_c(nc, tc, ctx, x1, attn, u_out):
    """U = conv(x1, attn) in natural [cout, pixel] layout."""
    cbpool = ctx.enter_context(tc.tile_pool(name="cbands", bufs=1))
    spool = ctx.enter_context(tc.tile_pool(name="stage", bufs=2))
    cps = ctx.enter_context(tc.tile_pool(name="cps", bufs=2, space="PSUM"))

    CBR = 18
    cb = [cbpool.tile([128, 2, CBR, 98], F32R, tag=f"cb{i}", name=f"cb{i}") for i in range(2)]
    for bt in cb:
        nc.gpsimd.memset(bt[:].bitcast(F32), 0.0)

    for g in range(6):
        band = cb[g % 2]
        y0 = 16 * g - 1
        r0 = max(0, -y0)
        r1 = min(CBR, 96 - y0)
        if g == 5:
            nc.gpsimd.memset(band[:, :, CBR - 1, :].bitcast(F32), 0.0)
        for j in range(2):
            nc.sync.dma_start(band[:, j, r0:r1, 1:97],
                              x1[j * 128:(j + 1) * 128, y0 + r0:y0 + r1, :])
        for i in range(2):
            ps = cps.tile([128, 4, 512], F32, tag="cps")
            idx = 0
            for t in range(K2):
                ty, tx = t // 3, t % 3
                for j in range(2):
                    lhsT = attn[:, j, t * COUT + i * 128: t * COUT + (i + 1) * 128]
                    st = (idx == 0)
                    sp = (idx == 2 * K2 - 1)
                    for b4 in range(4):
                        rr = 4 * b4 + ty
                        rhs = band[:, j, rr:rr + 4, tx:tx + 96]
                        nc.tensor.matmul(ps[:, b4, 0:384], lhsT, rhs, start=st, stop=sp)
                    idx += 1
            stage = spool.tile([128, 1536], F32, tag="stage")
            nc.vector.tensor_copy(stage[:], ps[:, :, 0:384])
            nc.sync.dma_start(u_out[i * 128:(i + 1) * 128, 16 * g * 96:(16 * g + 16) * 96],
                              stage[:])


def _emit(nc, tc, x1, x2, wkq1, wq2, pos1t, pos2t, u_out, stats_out):
    with tc.tile_pool(name="accum", bufs=1) as apool:
        a1t = apool.tile([128, 2, K2 * COUT], F32, tag="a1t")
        a2t = apool.tile([128, 2, K2 * COUT], F32, tag="a2t")

        with ExitStack() as ctx:
            _phase_b(nc, tc, ctx, x1, x2, wkq1, wq2, a1t, a2t, pos1t, pos2t)

        with ExitStack() as ctx2:
            atpool = ctx2.enter_context(tc.tile_pool(name="attn", bufs=1))
            spool2 = ctx2.enter_context(tc.tile_pool(name="stats", bufs=1))

            attn = atpool.tile([128, 2, K2 * COUT], F32R, tag="attn")
            nc.vector.tensor_tensor(out=attn[:], in0=a1t[:], in1=a2t[:], op=mybir.AluOpType.mult)

            attn_f = attn[:].bitcast(F32)
            stat = spool2.tile([128, 2], F32, tag="stat")
            nc.vector.reduce_sum(stat[:, 0:1], attn_f, axis=mybir.AxisListType.XY)
            sq = spool2.tile([128, 2, K2 * COUT], F32, tag="sq")
            nc.scalar.activation(sq[:], attn_f, mybir.ActivationFunctionType.Square,
                                 accum_out=stat[:, 1:2])
            nc.sync.dma_start(stats_out[:], stat[:])

            with ExitStack() as ctx3:
                _phase_c(nc, tc, ctx3, x1, attn, u_out)

def _prep_w(w):
    # [Cout, Cin, 3, 3] -> [cin, (ty*3+tx)*256 + cout]
    return np.ascontiguousarray(
        w.transpose(1, 2, 3, 0).reshape(CIN, K2 * COUT), dtype=np.float32)


def _prep_pos(pos):
    # [Cout, Cin*9] -> [cin, p*256 + cout]
    return np.ascontiguousarray(
        pos.reshape(COUT, CIN, K2).transpose(1, 2, 0).reshape(CIN, K2 * COUT),
        dtype=np.float32)


def build_in_maps(x1, x2, key_w, q1_w, q2_w, pos1, pos2):
    wk_r, wq1_r, wq2_r = _prep_w(key_w), _prep_w(q1_w), _prep_w(q2_w)
    wkq1_r = np.ascontiguousarray(
        np.concatenate([wk_r.reshape(CIN, K2, COUT), wq1_r.reshape(CIN, K2, COUT)],
                       axis=2).reshape(CIN, K2 * 2 * COUT))
    p1_r, p2_r = _prep_pos(pos1), _prep_pos(pos2)
    return [{
        "x1": np.ascontiguousarray(x1[b]),
        "x2": np.ascontiguousarray(x2[b]),
        "wkq1": wkq1_r, "wq2": wq2_r,
        "pos1t": p1_r, "pos2t": p2_r,
    } for b in range(N_CORES)]


def kernel(**inputs):
    x1 = np.asarray(inputs["x1"], dtype=np.float32)
    x2 = np.asarray(inputs["x2"], dtype=np.float32)
    key_w = np.asarray(inputs["key_w"], dtype=np.float32)
    q1_w = np.asarray(inputs["q1_w"], dtype=np.float32)
    q2_w = np.asarray(inputs["q2_w"], dtype=np.float32)
    pos1 = np.asarray(inputs["pos1"], dtype=np.float32)
    pos2 = np.asarray(inputs["pos2"], dtype=np.float32)
    momentum = float(np.asarray(inputs["kernel_momentum"]))

    assert x1.shape == (B, CIN, H, W), x1.shape

    if "nc" not in _CACHE:
        _CACHE["nc"] = build_nc()
    nc = _CACHE["nc"]

    in_maps = build_in_maps(x1, x2, key_w, q1_w, q2_w, pos1, pos2)
    _CACHE["last_in_maps"] = in_maps
    res = run_bass_kernel_spmd(nc, in_maps, list(range(N_CORES)))

    U = np.stack([res.results[b]["u"] for b in range(N_CORES)])          # [B, Cout, HW]
    stats = np.stack([res.results[b]["stats"] for b in range(N_CORES)])  # [B, 128, 2]

    s1 = float(stats[:, :, 0].astype(np.float64).sum())
    s2 = float(stats[:, :, 1].astype(np.float64).sum())
    n = float(B * COUT * CIN * K2)
    mu = s1 / n
    var = (s2 - s1 * s1 / n) / (n - 1.0)
    sd = np.sqrt(max(var, 0.0))
    scale = NORM_SCALE / (sd + 1e-4)
    a = momentum + scale
    bb = -mu * scale

    # boxsum(x1): conv(x1, all-ones weights) — identical for every out channel
    t = x1.sum(axis=1)                                   # [B, 96, 96]
    tp = np.pad(t, ((0, 0), (1, 1), (1, 1)))
    s = np.zeros_like(t)
    for ty in range(3):
        for tx in range(3):
            s += tp[:, ty:ty + 96, tx:tx + 96]

    out = a * U.reshape(B, COUT, H, W) + bb * s[:, None, :, :]
    return out.astype(np.float32)


# revision 17
# speedup vs baseline: 1.0969x; 1.0969x over previous
"""AttnConv2d Trainium2 kernel (8-core SPMD, data-parallel over batch).

Problem (per sample b):
  CK  = conv3x3_same(x1, key_w);  CQ1 = conv3x3_same(x1, q1_w);  CQ2 = conv3x3_same(x2, q2_w)
  For each kernel position p (3x3 grid phase) the conv outputs are unfolded with
  stride 3 -> per-p pixel sets of L=1024 pixels.
  A1[o,c,p] = sum_l CQ1[o,pix] * CK[c,pix];  A2 likewise with CQ2.
  attn = (A1+pos1)*(A2+pos2);  global mean/std normalize (over the whole batch);
  out  = conv3x3_same(x1, attn reshaped to [Cout,Cin,3,3] per sample).

Device strategy (one sample per NeuronCore):
  - All matmuls in float32r (fp32 storage, ~1e-4 matmul rel err, 4x fp32 speed).
  - Convs computed in transposed layout [pixel, channel] (lhsT = shifted image
    band, rhs = weights), tiled so each 128-pixel tile belongs to one kernel
    position p; A accumulated as A^T[c, p*256+o] via PSUM + DVE adds.
  - Normalization is deferred via linearity: the device returns the
    UN-normalized U = conv(x1, attn_raw) plus per-core attn sum/sumsq partials.
    Host computes global a,b and finishes out = a*U + b*boxsum(x1) (boxsum is
    conv(x1, ones) = the contribution of the constant shift b to the conv).
"""

import os
import sys
from contextlib import ExitStack

import numpy as np

for _p in ("/opt/trn_rl_repo",):
    if _p not in sys.path and os.path.isdir(_p):
        sys.path.append(_p)

import concourse.bacc as bacc
import concourse.tile as tile
import concourse.mybir as mybir
from concourse.bass_utils import run_bass_kernel_spmd

F32 = mybir.dt.float32
F32R = mybir.dt.float32r

B, CIN, COUT, H, W = 8, 256, 256, 96, 96
K = 3
K2 = K * K
HW = H * W          # 9216
NORM_SCALE = 1.0
N_CORES = 8

_CACHE = {}


def build_nc():
    nc = bacc.Bacc("TRN2", target_bir_lowering=False, debug=False,
                   enable_asserts=True, num_devices=N_CORES)
    x1 = nc.dram_tensor("x1", [CIN, H, W], F32R, kind="ExternalInput")
    x2 = nc.dram_tensor("x2", [CIN, H, W], F32R, kind="ExternalInput")
    wkq1 = nc.dram_tensor("wkq1", [CIN, K2 * 2 * COUT], F32R, kind="ExternalInput")
    wq2 = nc.dram_tensor("wq2", [CIN, K2 * COUT], F32R, kind="ExternalInput")
    pos1t = nc.dram_tensor("pos1t", [CIN, K2 * COUT], F32, kind="ExternalInput")
    pos2t = nc.dram_tensor("pos2t", [CIN, K2 * COUT], F32, kind="ExternalInput")
    u_out = nc.dram_tensor("u", [COUT, HW], F32, kind="ExternalOutput")
    stats_out = nc.dram_tensor("stats", [128, 2], F32, kind="ExternalOutput")

    with tile.TileContext(nc) as tc:
        _emit(nc, tc, x1, x2, wkq1, wq2, pos1t, pos2t, u_out, stats_out)
    nc.compile()
    return nc


def _strided_view(ap, free_dims, offset_elems):
    """Custom strided free-dim view of an SBUF AP (keeps partition dim)."""
    v = ap.copy()
    a = v.ap
    while len(a) > 1:
        a.pop()
    for d in free_dims:
        a.append(list(d))
    v.ap = a
    v.offset = ap.offset + offset_elems
    return v


# lhsT block-start row in the per-phase band for tap offset v = dy+ty-1:
#   dy'=v%3 phase blocks: dy'=0 at rows 0..4 (ly' 4m..4m+4), dy'=1 at rows
#   5..8 (ly' 4m..4m+3), dy'=2 at rows 9..13 (ly' 4m-1..4m+3)
_SROW = {-1: 9, 0: 0, 1: 5, 2: 10, 3: 1}
# per-phase (dyp -> (block base, n rows, staging row of local r=0, ...)):
#   staging row of phase-local row r:  dy'=0: 3r+1, dy'=1: 3r+2, dy'=2: 3r
_PHASE_ROWS = {0: (0, 5, 1), 1: (5, 4, 2), 2: (9, 5, 0)}


def _phase_b(nc, tc, ctx, x1, x2, wkq1, wq2, a1t, a2t, pos1t, pos2t):
    """Convs in [pixel, channel] layout + A^T accumulation.

    Rowwise staging band (zero-padded 98-wide) is DMA'd from DRAM, then
    gpsimd copies split it into per-phase grids:
      band[c, j, dxx, drow, lx] = x[j*128+c, y(drow), 3*lx + (dxx-1)]
    so a tap (p=(dy,dx), t=(ty,tx)) reads the CONTIGUOUS 128-px block
    band[:, j, dx+tx, SROW[dy+ty-1] : +4, :]  (4 subgrid rows x 32).
    """
    wpool = ctx.enter_context(tc.tile_pool(name="weights", bufs=1))
    stpool = ctx.enter_context(tc.tile_pool(name="staging", bufs=1))
    bpool = ctx.enter_context(tc.tile_pool(name="bands", bufs=1))
    cpool = ctx.enter_context(tc.tile_pool(name="convsb", bufs=2))
    convps = ctx.enter_context(tc.tile_pool(name="convps", bufs=2, space="PSUM"))
    aps = ctx.enter_context(tc.tile_pool(name="aps", bufs=2, space="PSUM"))

    SR = 14
    st1 = stpool.tile([128, 2, SR, 98], F32R, tag="st1")
    st2 = stpool.tile([128, 2, SR, 98], F32R, tag="st2")
    nc.gpsimd.memset(st1[:].bitcast(F32), 0.0)
    nc.gpsimd.memset(st2[:].bitcast(F32), 0.0)
    # m=0 staging prefetch BEFORE the weight DMAs so the first split copies
    # (startup critical path) are not stuck behind ~7MB of weights in-queue
    for stg, xdram in ((st1, x1), (st2, x2)):
        for j in range(2):
            nc.sync.dma_start(stg[:, j, 1:SR, 1:97],
                              xdram[j * 128:(j + 1) * 128, 0:SR - 1, :])

    # wkq1: [cin, tau, 512] = [wk_tau | wq1_tau] packed on host.
    # Chunked per (j, tap) so the first conv matmuls only wait on one piece.
    wkq1_t = wpool.tile([128, 2, K2 * 2 * COUT], F32R, tag="wkq1")
    wq2_t = wpool.tile([128, 2, K2 * COUT], F32R, tag="wq2")
    for t in range(K2):
        for j in range(2):
            nc.sync.dma_start(wkq1_t[:, j, t * 512:(t + 1) * 512],
                              wkq1[j * 128:(j + 1) * 128, t * 512:(t + 1) * 512])
            nc.sync.dma_start(wq2_t[:, j, t * COUT:(t + 1) * COUT],
                              wq2[j * 128:(j + 1) * 128, t * COUT:(t + 1) * COUT])
    x1b = [bpool.tile([128, 2, 5, SR, 32], F32R, tag=f"x1b{i}", name=f"x1b{i}") for i in range(2)]
    x2b = [bpool.tile([128, 2, 5, SR, 32], F32R, tag=f"x2b{i}", name=f"x2b{i}") for i in range(2)]

    for m in range(8):
        bx1, bx2 = x1b[m % 2], x2b[m % 2]
        y0 = 12 * m - 1                       # img row of staging row 0
        r0, r1 = max(0, -y0), min(SR, 96 - y0)
        if m == 7:
            # m=6 filled staging row 13 with real data; y=96 must be zero
            nc.gpsimd.memset(st1[:, :, SR - 1, :].bitcast(F32), 0.0)
            nc.gpsimd.memset(st2[:, :, SR - 1, :].bitcast(F32), 0.0)
        if m > 0:
            for stg, xdram in ((st1, x1), (st2, x2)):
                for j in range(2):
                    nc.sync.dma_start(stg[:, j, r0:r1, 1:97],
                                      xdram[j * 128:(j + 1) * 128, y0 + r0:y0 + r1, :])
        # (staging for m=0 was prefetched before the weights)
        # phase-split copies: staging -> band, spread across idle engines
        # band[c, j, dxx, base+r, lx] = staging[c, j, srow0+3r, 3*lx + dxx]
        idx = 0
        for stg, band in ((st1, bx1), (st2, bx2)):
            for j in range(2):
                sv = stg[:, j, :, :]           # [128, 14, 98]
                for dyp in range(3):
                    base, nr, srow0 = _PHASE_ROWS[dyp]
                    dst = band[:, j, :, base:base + nr, :]         # [128, 5, nr, 32]
                    src = _strided_view(sv, [[1, 5], [3 * 98, nr], [3, 32]],
                                        srow0 * 98)
                    eng = (nc.scalar, nc.scalar, nc.gpsimd, nc.vector,
                           nc.scalar, nc.scalar, nc.gpsimd, nc.vector,
                           nc.scalar, nc.scalar, nc.gpsimd, nc.scalar)[idx]
                    if eng is nc.scalar:
                        nc.scalar.copy(dst, src)
                    else:
                        eng.tensor_copy(dst, src)
                    idx += 1
        if m == 0:
            # initialize accumulators with pos (A_final = A_raw + pos for
            # free); emitted after the startup-critical DMAs, sync queues
            for j in range(2):
                nc.sync.dma_start(a1t[:, j, :], pos1t[j * 128:(j + 1) * 128, :])
                nc.sync.dma_start(a2t[:, j, :], pos2t[j * 128:(j + 1) * 128, :])
        for p in range(K2):
            dy, dx = p // 3, p % 3
            ckq1_ps = convps.tile([128, 2 * COUT], F32, tag="ckq1_ps")
            q2_ps = convps.tile([128, COUT], F32, tag="q2_ps")
            for j in range(2):
                for t in range(K2):
                    ty, tx = t // 3, t % 3
                    v = dy + ty - 1
                    s = _SROW[v]
                    dxx = dx + tx            # (dx+tx-1) + 1
                    lhs1 = bx1[:, j, dxx, s:s + 4, :]
                    lhs2 = bx2[:, j, dxx, s:s + 4, :]
                    st = (j == 0 and t == 0)
                    sp = (j == 1 and t == K2 - 1)
                    nc.tensor.matmul(ckq1_ps[:], lhs1,
                                     wkq1_t[:, j, t * 512:(t + 1) * 512], start=st, stop=sp)
                    nc.tensor.matmul(q2_ps[:], lhs2,
                                     wq2_t[:, j, t * COUT:(t + 1) * COUT], start=st, stop=sp)
            ckt = cpool.tile([128, COUT], F32R, tag="ckt")
            q12t = cpool.tile([128, 2 * COUT], F32R, tag="q12t")
            nc.vector.tensor_copy(ckt[:], ckq1_ps[:, 0:COUT])
            nc.vector.tensor_copy(q12t[:, 0:COUT], ckq1_ps[:, COUT:2 * COUT])
            nc.vector.tensor_copy(q12t[:, COUT:2 * COUT], q2_ps[:])

            a12_ps = aps.tile([128, 2, 512], F32, tag="a12_ps")
            for j in range(2):
                nc.tensor.matmul(a12_ps[:, j, :], ckt[:, j * 128:(j + 1) * 128], q12t[:],
                                 start=True, stop=True)
            psl = slice(p * COUT, (p + 1) * COUT)
            nc.vector.tensor_tensor(out=a1t[:, :, psl], in0=a1t[:, :, psl],
                                    in1=a12_ps[:, :, 0:COUT], op=mybir.AluOpType.add)
            nc.vector.tensor_tensor(out=a2t[:, :, psl], in0=a2t[:, :, psl],
                                    in1=a12_ps[:, :, COUT:2 * COUT], op=mybir.AluOpType.add)


def _phase_c(nc, tc, ctx, x1, attn, u_out):
    """U = conv(x1, attn) in natural [cout, pixel] layout."""
    cbpool = ctx.enter_context(tc.tile_pool(name="cbands", bufs=1))
    spool = ctx.enter_context(tc.tile_pool(name="stage", bufs=2))
    cps = ctx.enter_context(tc.tile_pool(name="cps", bufs=2, space="PSUM"))

    CBR = 18
    cb = [cbpool.tile([128, 2, CBR, 98], F32R, tag=f"cb{i}", name=f"cb{i}") for i in range(2)]
    for bt in cb:
        nc.gpsimd.memset(bt[:].bitcast(F32), 0.0)

    for g in range(6):
        band = cb[g % 2]
        y0 = 16 * g - 1
        r0 = max(0, -y0)
        r1 = min(CBR, 96 - y0)
        if g == 5:
            nc.gpsimd.memset(band[:, :, CBR - 1, :].bitcast(F32), 0.0)
        for j in range(2):
            nc.sync.dma_start(band[:, j, r0:r1, 1:97],
                              x1[j * 128:(j + 1) * 128, y0 + r0:y0 + r1, :])
        for i in range(2):
            ps = cps.tile([128, 4, 512], F32, tag="cps")
            idx = 0
            for t in range(K2):
                ty, tx = t // 3, t % 3
                for j in range(2):
                    lhsT = attn[:, j, t * COUT + i * 128: t * COUT + (i + 1) * 128]
                    st = (idx == 0)
                    sp = (idx == 2 * K2 - 1)
                    for b4 in range(4):
                        rr = 4 * b4 + ty
                        rhs = band[:, j, rr:rr + 4, tx:tx + 96]
                        nc.tensor.matmul(ps[:, b4, 0:384], lhsT, rhs, start=st, stop=sp)
                    idx += 1
            stage = spool.tile([128, 1536], F32, tag="stage")
            nc.vector.tensor_copy(stage[:], ps[:, :, 0:384])
            nc.sync.dma_start(u_out[i * 128:(i + 1) * 128, 16 * g * 96:(16 * g + 16) * 96],
                              stage[:])


def _emit(nc, tc, x1, x2, wkq1, wq2, pos1t, pos2t, u_out, stats_out):
    with tc.tile_pool(name="accum", bufs=1) as apool:
        a1t = apool.tile([128, 2, K2 * COUT], F32, tag="a1t")
        a2t = apool.tile([128, 2, K2 * COUT], F32, tag="a2t")

        with ExitStack() as ctx:
            _phase_b(nc, tc, ctx, x1, x2, wkq1, wq2, a1t, a2t, pos1t, pos2t)

        with ExitStack() as ctx2:
            atpool = ctx2.enter_context(tc.tile_pool(name="attn", bufs=1))
            spool2 = ctx2.enter_context(tc.tile_pool(name="stats", bufs=1))

            attn = atpool.tile([128, 2, K2 * COUT], F32R, tag="attn")
            nc.vector.tensor_tensor(out=attn[:], in0=a1t[:], in1=a2t[:], op=mybir.AluOpType.mult)

            attn_f = attn[:].bitcast(F32)
            stat = spool2.tile([128, 2], F32, tag="stat")
            nc.vector.reduce_sum(stat[:, 0:1], attn_f, axis=mybir.AxisListType.XY)
            sq = spool2.tile([128, 2, K2 * COUT], F32, tag="sq")
            nc.scalar.activation(sq[:], attn_f, mybir.ActivationFunctionType.Square,
                                 accum_out=stat[:, 1:2])
            nc.sync.dma_start(stats_out[:], stat[:])

            with ExitStack() as ctx3:
                _phase_c(nc, tc, ctx3, x1, attn, u_out)

def _prep_w(w):
    # [Cout, Cin, 3, 3] -> [cin, (ty*3+tx)*256 + cout]
    return np.ascontiguousarray(
        w.transpose(1, 2, 3, 0).reshape(CIN, K2 * COUT), dtype=np.float32)


def _prep_pos(pos):
    # [Cout, Cin*9] -> [cin, p*256 + cout]
    return np.ascontiguousarray(
        pos.reshape(COUT, CIN, K2).transpose(1, 2, 0).reshape(CIN, K2 * COUT),
        dtype=np.float32)


def build_in_maps(x1, x2, key_w, q1_w, q2_w, pos1, pos2):
    wk_r, wq1_r, wq2_r = _prep_w(key_w), _prep_w(q1_w), _prep_w(q2_w)
    wkq1_r = np.ascontiguousarray(
        np.concatenate([wk_r.reshape(CIN, K2, COUT), wq1_r.reshape(CIN, K2, COUT)],
                       axis=2).reshape(CIN, K2 * 2 * COUT))
    p1_r, p2_r = _prep_pos(pos1), _prep_pos(pos2)
    return [{
        "x1": np.ascontiguousarray(x1[b]),
        "x2": np.ascontiguousarray(x2[b]),
        "wkq1": wkq1_r, "wq2": wq2_r,
        "pos1t": p1_r, "pos2t": p2_r,
    } for b in range(N_CORES)]


def kernel(**inputs):
    x1 = np.asarray(inputs["x1"], dtype=np.float32)
    x2 = np.asarray(inputs["x2"], dtype=np.float32)
    key_w = np.asarray(inputs["key_w"], dtype=np.float32)
    q1_w = np.asarray(inputs["q1_w"], dtype=np.float32)
    q2_w = np.asarray(inputs["q2_w"], dtype=np.float32)
    pos1 = np.asarray(inputs["pos1"], dtype=np.float32)
    pos2 = np.asarray(inputs["pos2"], dtype=np.float32)
    momentum = float(np.asarray(inputs["kernel_momentum"]))

    assert x1.shape == (B, CIN, H, W), x1.shape

    if "nc" not in _CACHE:
        _CACHE["nc"] = build_nc()
    nc = _CACHE["nc"]

    in_maps = build_in_maps(x1, x2, key_w, q1_w, q2_w, pos1, pos2)
    _CACHE["last_in_maps"] = in_maps
    res = run_bass_kernel_spmd(nc, in_maps, list(range(N_CORES)))

    U = np.stack([res.results[b]["u"] for b in range(N_CORES)])          # [B, Cout, HW]
    stats = np.stack([res.results[b]["stats"] for b in range(N_CORES)])  # [B, 128, 2]

    s1 = float(stats[:, :, 0].astype(np.float64).sum())
    s2 = float(stats[:, :, 1].astype(np.float64).sum())
    n = float(B * COUT * CIN * K2)
    mu = s1 / n
    var = (s2 - s1 * s1 / n) / (n - 1.0)
    sd = np.sqrt(max(var, 0.0))
    scale = NORM_SCALE / (sd + 1e-4)
    a = momentum + scale
    bb = -mu * scale

    # boxsum(x1): conv(x1, all-ones weights) — identical for every out channel
    t = x1.sum(axis=1)                                   # [B, 96, 96]
    tp = np.pad(t, ((0, 0), (1, 1), (1, 1)))
    s = np.zeros_like(t)
    for ty in range(3):
        for tx in range(3):
            s += tp[:, ty:ty + 96, tx:tx + 96]

    out = a * U.reshape(B, COUT, H, W) + bb * s[:, None, :, :]
    return out.astype(np.float32)


# revision 18
# speedup vs baseline: 1.2026x; 1.0964x over previous
"""AttnConv2d Trainium2 kernel (8-core SPMD, data-parallel over batch).

Problem (per sample b):
  CK  = conv3x3_same(x1, key_w);  CQ1 = conv3x3_same(x1, q1_w);  CQ2 = conv3x3_same(x2, q2_w)
  For each kernel position p (3x3 grid phase) the conv outputs are unfolded with
  stride 3 -> per-p pixel sets of L=1024 pixels.
  A1[o,c,p] = sum_l CQ1[o,pix] * CK[c,pix];  A2 likewise with CQ2.
  attn = (A1+pos1)*(A2+pos2);  global mean/std normalize (over the whole batch);
  out  = conv3x3_same(x1, attn reshaped to [Cout,Cin,3,3] per sample).

Device strategy (one sample per NeuronCore):
  - All matmuls in float32r (fp32 storage, ~1e-4 matmul rel err, 4x fp32 speed).
  - Convs computed in transposed layout [pixel, channel] (lhsT = shifted image
    band, rhs = weights), tiled so each 128-pixel tile belongs to one kernel
    position p; A accumulated as A^T[c, p*256+o] via PSUM + DVE adds.
  - Normalization is deferred via linearity: the device returns the
    UN-normalized U = conv(x1, attn_raw) plus per-core attn sum/sumsq partials.
    Host computes global a,b and finishes out = a*U + b*boxsum(x1) (boxsum is
    conv(x1, ones) = the contribution of the constant shift b to the conv).
"""

import os
import sys
from contextlib import ExitStack

import numpy as np

for _p in ("/opt/trn_rl_repo",):
    if _p not in sys.path and os.path.isdir(_p):
        sys.path.append(_p)

import concourse.bacc as bacc
import concourse.tile as tile
import concourse.mybir as mybir
from concourse.bass_utils import run_bass_kernel_spmd

F32 = mybir.dt.float32
F32R = mybir.dt.float32r

B, CIN, COUT, H, W = 8, 256, 256, 96, 96
K = 3
K2 = K * K
HW = H * W          # 9216
NORM_SCALE = 1.0
N_CORES = 8

_CACHE = {}


def build_nc():
    nc = bacc.Bacc("TRN2", target_bir_lowering=False, debug=False,
                   enable_asserts=True, num_devices=N_CORES)
    x1 = nc.dram_tensor("x1", [CIN, H, W], F32R, kind="ExternalInput")
    x2 = nc.dram_tensor("x2", [CIN, H, W], F32R, kind="ExternalInput")
    wkq1 = nc.dram_tensor("wkq1", [CIN, K2 * 2 * COUT], F32R, kind="ExternalInput")
    wq2 = nc.dram_tensor("wq2", [CIN, K2 * COUT], F32R, kind="ExternalInput")
    pos1t = nc.dram_tensor("pos1t", [CIN, K2 * COUT], F32, kind="ExternalInput")
    pos2t = nc.dram_tensor("pos2t", [CIN, K2 * COUT], F32, kind="ExternalInput")
    u_out = nc.dram_tensor("u", [COUT, HW], F32, kind="ExternalOutput")
    stats_out = nc.dram_tensor("stats", [128, 2], F32, kind="ExternalOutput")

    with tile.TileContext(nc) as tc:
        _emit(nc, tc, x1, x2, wkq1, wq2, pos1t, pos2t, u_out, stats_out)
    nc.compile()
    return nc


def _strided_view(ap, free_dims, offset_elems):
    """Custom strided free-dim view of an SBUF AP (keeps partition dim)."""
    v = ap.copy()
    a = v.ap
    while len(a) > 1:
        a.pop()
    for d in free_dims:
        a.append(list(d))
    v.ap = a
    v.offset = ap.offset + offset_elems
    return v


# lhsT block-start row in the per-phase band for tap offset v = dy+ty-1:
#   dy'=v%3 phase blocks: dy'=0 at rows 0..4 (ly' 4m..4m+4), dy'=1 at rows
#   5..8 (ly' 4m..4m+3), dy'=2 at rows 9..13 (ly' 4m-1..4m+3)
_SROW = {-1: 9, 0: 0, 1: 5, 2: 10, 3: 1}
# per-phase (dyp -> (block base, n rows, staging row of local r=0, ...)):
#   staging row of phase-local row r:  dy'=0: 3r+1, dy'=1: 3r+2, dy'=2: 3r
_PHASE_ROWS = {0: (0, 5, 1), 1: (5, 4, 2), 2: (9, 5, 0)}


def _phase_b(nc, tc, ctx, x1, x2, wkq1, wq2, a1t, a2t, pos1t, pos2t):
    """Convs in [pixel, channel] layout + A^T accumulation.

    Rowwise staging band (zero-padded 98-wide) is DMA'd from DRAM, then
    gpsimd copies split it into per-phase grids:
      band[c, j, dxx, drow, lx] = x[j*128+c, y(drow), 3*lx + (dxx-1)]
    so a tap (p=(dy,dx), t=(ty,tx)) reads the CONTIGUOUS 128-px block
    band[:, j, dx+tx, SROW[dy+ty-1] : +4, :]  (4 subgrid rows x 32).
    """
    wpool = ctx.enter_context(tc.tile_pool(name="weights", bufs=1))
    stpool = ctx.enter_context(tc.tile_pool(name="staging", bufs=1))
    bpool = ctx.enter_context(tc.tile_pool(name="bands", bufs=1))
    cpool = ctx.enter_context(tc.tile_pool(name="convsb", bufs=2))
    convps = ctx.enter_context(tc.tile_pool(name="convps", bufs=2, space="PSUM"))
    aps = ctx.enter_context(tc.tile_pool(name="aps", bufs=2, space="PSUM"))

    SR = 14
    st1 = stpool.tile([128, 2, SR, 98], F32R, tag="st1")
    st2 = stpool.tile([128, 2, SR, 98], F32R, tag="st2")
    nc.gpsimd.memset(st1[:].bitcast(F32), 0.0)
    nc.gpsimd.memset(st2[:].bitcast(F32), 0.0)
    # wkq1: [cin, tau, 512] = [wk_tau | wq1_tau] packed on host.
    # Chunked per (j, tap) so the first conv matmuls only wait on one piece.
    wkq1_t = wpool.tile([128, 2, K2 * 2 * COUT], F32R, tag="wkq1")
    wq2_t = wpool.tile([128, 2, K2 * COUT], F32R, tag="wq2")
    for t in range(K2):
        for j in range(2):
            nc.sync.dma_start(wkq1_t[:, j, t * 512:(t + 1) * 512],
                              wkq1[j * 128:(j + 1) * 128, t * 512:(t + 1) * 512])
            nc.sync.dma_start(wq2_t[:, j, t * COUT:(t + 1) * COUT],
                              wq2[j * 128:(j + 1) * 128, t * COUT:(t + 1) * COUT])
    x1b = [bpool.tile([128, 2, 5, SR, 32], F32R, tag=f"x1b{i}", name=f"x1b{i}") for i in range(2)]
    x2b = [bpool.tile([128, 2, 5, SR, 32], F32R, tag=f"x2b{i}", name=f"x2b{i}") for i in range(2)]

    for m in range(8):
        bx1, bx2 = x1b[m % 2], x2b[m % 2]
        y0 = 12 * m - 1                       # img row of staging row 0
        r0, r1 = max(0, -y0), min(SR, 96 - y0)
        if m == 7:
            # m=6 filled staging row 13 with real data; y=96 must be zero
            nc.gpsimd.memset(st1[:, :, SR - 1, :].bitcast(F32), 0.0)
            nc.gpsimd.memset(st2[:, :, SR - 1, :].bitcast(F32), 0.0)
        for stg, xdram in ((st1, x1), (st2, x2)):
            for j in range(2):
                nc.sync.dma_start(stg[:, j, r0:r1, 1:97],
                                  xdram[j * 128:(j + 1) * 128, y0 + r0:y0 + r1, :])
        # phase-split copies: staging -> band, spread across idle engines
        # band[c, j, dxx, base+r, lx] = staging[c, j, srow0+3r, 3*lx + dxx]
        idx = 0
        for stg, band in ((st1, bx1), (st2, bx2)):
            for j in range(2):
                sv = stg[:, j, :, :]           # [128, 14, 98]
                for dyp in range(3):
                    base, nr, srow0 = _PHASE_ROWS[dyp]
                    dst = band[:, j, :, base:base + nr, :]         # [128, 5, nr, 32]
                    src = _strided_view(sv, [[1, 5], [3 * 98, nr], [3, 32]],
                                        srow0 * 98)
                    eng = (nc.scalar, nc.scalar, nc.gpsimd, nc.vector,
                           nc.scalar, nc.scalar, nc.gpsimd, nc.vector,
                           nc.scalar, nc.scalar, nc.gpsimd, nc.scalar)[idx]
                    if eng is nc.scalar:
                        nc.scalar.copy(dst, src)
                    else:
                        eng.tensor_copy(dst, src)
                    idx += 1
        if m == 0:
            # initialize accumulators with pos (A_final = A_raw + pos for
            # free); emitted after the startup-critical DMAs, sync queues
            for j in range(2):
                nc.sync.dma_start(a1t[:, j, :], pos1t[j * 128:(j + 1) * 128, :])
                nc.sync.dma_start(a2t[:, j, :], pos2t[j * 128:(j + 1) * 128, :])
        for p in range(K2):
            dy, dx = p // 3, p % 3
            ckq1_ps = convps.tile([128, 2 * COUT], F32, tag="ckq1_ps")
            q2_ps = convps.tile([128, COUT], F32, tag="q2_ps")
            for j in range(2):
                for t in range(K2):
                    ty, tx = t // 3, t % 3
                    v = dy + ty - 1
                    s = _SROW[v]
                    dxx = dx + tx            # (dx+tx-1) + 1
                    lhs1 = bx1[:, j, dxx, s:s + 4, :]
                    lhs2 = bx2[:, j, dxx, s:s + 4, :]
                    st = (j == 0 and t == 0)
                    sp = (j == 1 and t == K2 - 1)
                    nc.tensor.matmul(ckq1_ps[:], lhs1,
                                     wkq1_t[:, j, t * 512:(t + 1) * 512], start=st, stop=sp)
                    nc.tensor.matmul(q2_ps[:], lhs2,
                                     wq2_t[:, j, t * COUT:(t + 1) * COUT], start=st, stop=sp)
            ckt = cpool.tile([128, COUT], F32R, tag="ckt")
            q12t = cpool.tile([128, 2 * COUT], F32R, tag="q12t")
            nc.vector.tensor_copy(ckt[:], ckq1_ps[:, 0:COUT])
            nc.vector.tensor_copy(q12t[:, 0:COUT], ckq1_ps[:, COUT:2 * COUT])
            nc.vector.tensor_copy(q12t[:, COUT:2 * COUT], q2_ps[:])

            a12_ps = aps.tile([128, 2, 512], F32, tag="a12_ps")
            for j in range(2):
                nc.tensor.matmul(a12_ps[:, j, :], ckt[:, j * 128:(j + 1) * 128], q12t[:],
                                 start=True, stop=True)
            psl = slice(p * COUT, (p + 1) * COUT)
            nc.vector.tensor_tensor(out=a1t[:, :, psl], in0=a1t[:, :, psl],
                                    in1=a12_ps[:, :, 0:COUT], op=mybir.AluOpType.add)
            nc.vector.tensor_tensor(out=a2t[:, :, psl], in0=a2t[:, :, psl],
                                    in1=a12_ps[:, :, COUT:2 * COUT], op=mybir.AluOpType.add)


def _phase_c(nc, tc, ctx, x1, attn, u_out):
    """U = conv(x1, attn) in natural [cout, pixel] layout."""
    cbpool = ctx.enter_context(tc.tile_pool(name="cbands", bufs=1))
    spool = ctx.enter_context(tc.tile_pool(name="stage", bufs=2))
    cps = ctx.enter_context(tc.tile_pool(name="cps", bufs=2, space="PSUM"))

    CBR = 18
    cb = [cbpool.tile([128, 2, CBR, 98], F32R, tag=f"cb{i}", name=f"cb{i}") for i in range(2)]
    for bt in cb:
        nc.gpsimd.memset(bt[:].bitcast(F32), 0.0)

    for g in range(6):
        band = cb[g % 2]
        y0 = 16 * g - 1
        r0 = max(0, -y0)
        r1 = min(CBR, 96 - y0)
        if g == 5:
            nc.gpsimd.memset(band[:, :, CBR - 1, :].bitcast(F32), 0.0)
        for j in range(2):
            nc.sync.dma_start(band[:, j, r0:r1, 1:97],
                              x1[j * 128:(j + 1) * 128, y0 + r0:y0 + r1, :])
        for i in range(2):
            ps = cps.tile([128, 4, 512], F32, tag="cps")
            idx = 0
            for t in range(K2):
                ty, tx = t // 3, t % 3
                for j in range(2):
                    lhsT = attn[:, j, t * COUT + i * 128: t * COUT + (i + 1) * 128]
                    st = (idx == 0)
                    sp = (idx == 2 * K2 - 1)
                    for b4 in range(4):
                        rr = 4 * b4 + ty
                        rhs = band[:, j, rr:rr + 4, tx:tx + 96]
                        nc.tensor.matmul(ps[:, b4, 0:384], lhsT, rhs, start=st, stop=sp)
                    idx += 1
            stage = spool.tile([128, 1536], F32, tag="stage")
            nc.vector.tensor_copy(stage[:], ps[:, :, 0:384])
            nc.sync.dma_start(u_out[i * 128:(i + 1) * 128, 16 * g * 96:(16 * g + 16) * 96],
                              stage[:])


def _emit(nc, tc, x1, x2, wkq1, wq2, pos1t, pos2t, u_out, stats_out):
    with tc.tile_pool(name="accum", bufs=1) as apool:
        a1t = apool.tile([128, 2, K2 * COUT], F32, tag="a1t")
        a2t = apool.tile([128, 2, K2 * COUT], F32, tag="a2t")

        with ExitStack() as ctx:
            _phase_b(nc, tc, ctx, x1, x2, wkq1, wq2, a1t, a2t, pos1t, pos2t)

        with ExitStack() as ctx2:
            atpool = ctx2.enter_context(tc.tile_pool(name="attn", bufs=1))
            spool2 = ctx2.enter_context(tc.tile_pool(name="stats", bufs=1))

            attn = atpool.tile([128, 2, K2 * COUT], F32R, tag="attn")
            nc.vector.tensor_tensor(out=attn[:], in0=a1t[:], in1=a2t[:], op=mybir.AluOpType.mult)

            attn_f = attn[:].bitcast(F32)
            stat = spool2.tile([128, 2], F32, tag="stat")
            nc.vector.reduce_sum(stat[:, 0:1], attn_f, axis=mybir.AxisListType.XY)
            sq = spool2.tile([128, 2, K2 * COUT], F32, tag="sq")
            nc.scalar.activation(sq[:], attn_f, mybir.ActivationFunctionType.Square,
                                 accum_out=stat[:, 1:2])
            nc.sync.dma_start(stats_out[:], stat[:])

            with ExitStack() as ctx3:
                _phase_c(nc, tc, ctx3, x1, attn, u_out)

def _prep_w(w):
    # [Cout, Cin, 3, 3] -> [cin, (ty*3+tx)*256 + cout]
    return np.ascontiguousarray(
        w.transpose(1, 2, 3, 0).reshape(CIN, K2 * COUT), dtype=np.float32)


def _prep_pos(pos):
    # [Cout, Cin*9] -> [cin, p*256 + cout]
    return np.ascontiguousarray(
        pos.reshape(COUT, CIN, K2).transpose(1, 2, 0).reshape(CIN, K2 * COUT),
        dtype=np.float32)


def build_in_maps(x1, x2, key_w, q1_w, q2_w, pos1, pos2):
    wk_r, wq1_r, wq2_r = _prep_w(key_w), _prep_w(q1_w), _prep_w(q2_w)
    wkq1_r = np.ascontiguousarray(
        np.concatenate([wk_r.reshape(CIN, K2, COUT), wq1_r.reshape(CIN, K2, COUT)],
                       axis=2).reshape(CIN, K2 * 2 * COUT))
    p1_r, p2_r = _prep_pos(pos1), _prep_pos(pos2)
    return [{
        "x1": np.ascontiguousarray(x1[b]),
        "x2": np.ascontiguousarray(x2[b]),
        "wkq1": wkq1_r, "wq2": wq2_r,
        "pos1t": p1_r, "pos2t": p2_r,
    } for b in range(N_CORES)]


def kernel(**inputs):
    x1 = np.asarray(inputs["x1"], dtype=np.float32)
    x2 = np.asarray(inputs["x2"], dtype=np.float32)
    key_w = np.asarray(inputs["key_w"], dtype=np.float32)
    q1_w = np.asarray(inputs["q1_w"], dtype=np.float32)
    q2_w = np.asarray(inputs["q2_w"], dtype=np.float32)
    pos1 = np.asarray(inputs["pos1"], dtype=np.float32)
    pos2 = np.asarray(inputs["pos2"], dtype=np.float32)
    momentum = float(np.asarray(inputs["kernel_momentum"]))

    assert x1.shape == (B, CIN, H, W), x1.shape

    if "nc" not in _CACHE:
        _CACHE["nc"] = build_nc()
    nc = _CACHE["nc"]

    in_maps = build_in_maps(x1, x2, key_w, q1_w, q2_w, pos1, pos2)
    _CACHE["last_in_maps"] = in_maps
    res = run_bass_kernel_spmd(nc, in_maps, list(range(N_CORES)))

    U = np.stack([res.results[b]["u"] for b in range(N_CORES)])          # [B, Cout, HW]
    stats = np.stack([res.results[b]["stats"] for b in range(N_CORES)])  # [B, 128, 2]

    s1 = float(stats[:, :, 0].astype(np.float64).sum())
    s2 = float(stats[:, :, 1].astype(np.float64).sum())
    n = float(B * COUT * CIN * K2)
    mu = s1 / n
    var = (s2 - s1 * s1 / n) / (n - 1.0)
    sd = np.sqrt(max(var, 0.0))
    scale = NORM_SCALE / (sd + 1e-4)
    a = momentum + scale
    bb = -mu * scale

    # boxsum(x1): conv(x1, all-ones weights) — identical for every out channel
    t = x1.sum(axis=1)                                   # [B, 96, 96]
    tp = np.pad(t, ((0, 0), (1, 1), (1, 1)))
    s = np.zeros_like(t)
    for ty in range(3):
        for tx in range(3):
            s += tp[:, ty:ty + 96, tx:tx + 96]

    out = a * U.reshape(B, COUT, H, W) + bb * s[:, None, :, :]
    return out.astype(np.float32)


# revision 19
# speedup vs baseline: 1.2055x; 1.0024x over previous
"""AttnConv2d Trainium2 kernel (8-core SPMD, data-parallel over batch).

Problem (per sample b):
  CK  = conv3x3_same(x1, key_w);  CQ1 = conv3x3_same(x1, q1_w);  CQ2 = conv3x3_same(x2, q2_w)
  For each kernel position p (3x3 grid phase) the conv outputs are unfolded with
  stride 3 -> per-p pixel sets of L=1024 pixels.
  A1[o,c,p] = sum_l CQ1[o,pix] * CK[c,pix];  A2 likewise with CQ2.
  attn = (A1+pos1)*(A2+pos2);  global mean/std normalize (over the whole batch);
  out  = conv3x3_same(x1, attn reshaped to [Cout,Cin,3,3] per sample).

Device strategy (one sample per NeuronCore):
  - All matmuls in float32r (fp32 storage, ~1e-4 matmul rel err, 4x fp32 speed).
  - Convs computed in transposed layout [pixel, channel] (lhsT = shifted image
    band, rhs = weights), tiled so each 128-pixel tile belongs to one kernel
    position p; A accumulated as A^T[c, p*256+o] via PSUM + DVE adds.
  - Normalization is deferred via linearity: the device returns the
    UN-normalized U = conv(x1, attn_raw) plus per-core attn sum/sumsq partials.
    Host computes global a,b and finishes out = a*U + b*boxsum(x1) (boxsum is
    conv(x1, ones) = the contribution of the constant shift b to the conv).
"""

import os
import sys
from contextlib import ExitStack

import numpy as np

for _p in ("/opt/trn_rl_repo",):
    if _p not in sys.path and os.path.isdir(_p):
        sys.path.append(_p)

import concourse.bacc as bacc
import concourse.tile as tile
import concourse.mybir as mybir
from concourse.bass_utils import run_bass_kernel_spmd

F32 = mybir.dt.float32
F32R = mybir.dt.float32r

B, CIN, COUT, H, W = 8, 256, 256, 96, 96
K = 3
K2 = K * K
HW = H * W          # 9216
NORM_SCALE = 1.0
N_CORES = 8

_CACHE = {}


def build_nc():
    nc = bacc.Bacc("TRN2", target_bir_lowering=False, debug=False,
                   enable_asserts=True, num_devices=N_CORES)
    x1 = nc.dram_tensor("x1", [CIN, H, W], F32R, kind="ExternalInput")
    x2 = nc.dram_tensor("x2", [CIN, H, W], F32R, kind="ExternalInput")
    wkq1 = nc.dram_tensor("wkq1", [CIN, K2 * 2 * COUT], F32R, kind="ExternalInput")
    wq2 = nc.dram_tensor("wq2", [CIN, K2 * COUT], F32R, kind="ExternalInput")
    pos1t = nc.dram_tensor("pos1t", [CIN, K2 * COUT], F32, kind="ExternalInput")
    pos2t = nc.dram_tensor("pos2t", [CIN, K2 * COUT], F32, kind="ExternalInput")
    u_out = nc.dram_tensor("u", [COUT, HW], F32, kind="ExternalOutput")
    stats_out = nc.dram_tensor("stats", [128, 2], F32, kind="ExternalOutput")

    with tile.TileContext(nc) as tc:
        _emit(nc, tc, x1, x2, wkq1, wq2, pos1t, pos2t, u_out, stats_out)
    nc.compile()
    return nc


def _strided_view(ap, free_dims, offset_elems):
    """Custom strided free-dim view of an SBUF AP (keeps partition dim)."""
    v = ap.copy()
    a = v.ap
    while len(a) > 1:
        a.pop()
    for d in free_dims:
        a.append(list(d))
    v.ap = a
    v.offset = ap.offset + offset_elems
    return v


# lhsT block-start row in the per-phase band for tap offset v = dy+ty-1:
#   dy'=v%3 phase blocks: dy'=0 at rows 0..4 (ly' 4m..4m+4), dy'=1 at rows
#   5..8 (ly' 4m..4m+3), dy'=2 at rows 9..13 (ly' 4m-1..4m+3)
_SROW = {-1: 9, 0: 0, 1: 5, 2: 10, 3: 1}
# per-phase (dyp -> (block base, n rows, staging row of local r=0, ...)):
#   staging row of phase-local row r:  dy'=0: 3r+1, dy'=1: 3r+2, dy'=2: 3r
_PHASE_ROWS = {0: (0, 5, 1), 1: (5, 4, 2), 2: (9, 5, 0)}


def _phase_b(nc, tc, ctx, x1, x2, wkq1, wq2, a1t, a2t, pos1t, pos2t):
    """Convs in [pixel, channel] layout + A^T accumulation.

    Rowwise staging band (zero-padded 98-wide) is DMA'd from DRAM, then
    gpsimd copies split it into per-phase grids:
      band[c, j, dxx, drow, lx] = x[j*128+c, y(drow), 3*lx + (dxx-1)]
    so a tap (p=(dy,dx), t=(ty,tx)) reads the CONTIGUOUS 128-px block
    band[:, j, dx+tx, SROW[dy+ty-1] : +4, :]  (4 subgrid rows x 32).
    """
    wpool = ctx.enter_context(tc.tile_pool(name="weights", bufs=1))
    stpool = ctx.enter_context(tc.tile_pool(name="staging", bufs=1))
    bpool = ctx.enter_context(tc.tile_pool(name="bands", bufs=1))
    cpool = ctx.enter_context(tc.tile_pool(name="convsb", bufs=2))
    convps = ctx.enter_context(tc.tile_pool(name="convps", bufs=2, space="PSUM"))
    aps = ctx.enter_context(tc.tile_pool(name="aps", bufs=2, space="PSUM"))

    SR = 14
    st1 = stpool.tile([128, 2, SR, 98], F32R, tag="st1")
    st2 = stpool.tile([128, 2, SR, 98], F32R, tag="st2")
    nc.gpsimd.memset(st1[:].bitcast(F32), 0.0)
    nc.gpsimd.memset(st2[:].bitcast(F32), 0.0)
    # wkq1: [cin, tau, 512] = [wk_tau | wq1_tau] packed on host.
    # Chunked per (j, tap) so the first conv matmuls only wait on one piece.
    wkq1_t = wpool.tile([128, 2, K2 * 2 * COUT], F32R, tag="wkq1")
    wq2_t = wpool.tile([128, 2, K2 * COUT], F32R, tag="wq2")
    # weights go on the Activation HWDGE queue so they stream concurrently
    # with the staging bands on the SP queue
    for t in range(K2):
        for j in range(2):
            nc.scalar.dma_start(wkq1_t[:, j, t * 512:(t + 1) * 512],
                                wkq1[j * 128:(j + 1) * 128, t * 512:(t + 1) * 512])
            nc.scalar.dma_start(wq2_t[:, j, t * COUT:(t + 1) * COUT],
                                wq2[j * 128:(j + 1) * 128, t * COUT:(t + 1) * COUT])
    x1b = [bpool.tile([128, 2, 5, SR, 32], F32R, tag=f"x1b{i}", name=f"x1b{i}") for i in range(2)]
    x2b = [bpool.tile([128, 2, 5, SR, 32], F32R, tag=f"x2b{i}", name=f"x2b{i}") for i in range(2)]

    for m in range(8):
        bx1, bx2 = x1b[m % 2], x2b[m % 2]
        y0 = 12 * m - 1                       # img row of staging row 0
        r0, r1 = max(0, -y0), min(SR, 96 - y0)
        if m == 7:
            # m=6 filled staging row 13 with real data; y=96 must be zero
            nc.gpsimd.memset(st1[:, :, SR - 1, :].bitcast(F32), 0.0)
            nc.gpsimd.memset(st2[:, :, SR - 1, :].bitcast(F32), 0.0)
        for stg, xdram in ((st1, x1), (st2, x2)):
            for j in range(2):
                nc.sync.dma_start(stg[:, j, r0:r1, 1:97],
                                  xdram[j * 128:(j + 1) * 128, y0 + r0:y0 + r1, :])
        # phase-split copies: staging -> band, spread across idle engines
        # band[c, j, dxx, base+r, lx] = staging[c, j, srow0+3r, 3*lx + dxx]
        idx = 0
        for stg, band in ((st1, bx1), (st2, bx2)):
            for j in range(2):
                sv = stg[:, j, :, :]           # [128, 14, 98]
                for dyp in range(3):
                    base, nr, srow0 = _PHASE_ROWS[dyp]
                    dst = band[:, j, :, base:base + nr, :]         # [128, 5, nr, 32]
                    src = _strided_view(sv, [[1, 5], [3 * 98, nr], [3, 32]],
                                        srow0 * 98)
                    if m == 0:
                        # ACT queue is busy issuing weights at startup
                        eng = (nc.vector, nc.vector, nc.gpsimd, nc.vector,
                               nc.vector, nc.gpsimd, nc.vector, nc.vector,
                               nc.gpsimd, nc.vector, nc.vector, nc.gpsimd)[idx]
                    else:
                        eng = (nc.scalar, nc.scalar, nc.gpsimd, nc.vector,
                               nc.scalar, nc.scalar, nc.gpsimd, nc.vector,
                               nc.scalar, nc.scalar, nc.gpsimd, nc.scalar)[idx]
                    if eng is nc.scalar:
                        nc.scalar.copy(dst, src)
                    else:
                        eng.tensor_copy(dst, src)
                    idx += 1
        if m == 0:
            # initialize accumulators with pos (A_final = A_raw + pos for
            # free); on the ACT queue behind the weights
            for j in range(2):
                nc.scalar.dma_start(a1t[:, j, :], pos1t[j * 128:(j + 1) * 128, :])
                nc.scalar.dma_start(a2t[:, j, :], pos2t[j * 128:(j + 1) * 128, :])
        for p in range(K2):
            dy, dx = p // 3, p % 3
            ckq1_ps = convps.tile([128, 2 * COUT], F32, tag="ckq1_ps")
            q2_ps = convps.tile([128, COUT], F32, tag="q2_ps")
            for j in range(2):
                for t in range(K2):
                    ty, tx = t // 3, t % 3
                    v = dy + ty - 1
                    s = _SROW[v]
                    dxx = dx + tx            # (dx+tx-1) + 1
                    lhs1 = bx1[:, j, dxx, s:s + 4, :]
                    lhs2 = bx2[:, j, dxx, s:s + 4, :]
                    st = (j == 0 and t == 0)
                    sp = (j == 1 and t == K2 - 1)
                    nc.tensor.matmul(ckq1_ps[:], lhs1,
                                     wkq1_t[:, j, t * 512:(t + 1) * 512], start=st, stop=sp)
                    nc.tensor.matmul(q2_ps[:], lhs2,
                                     wq2_t[:, j, t * COUT:(t + 1) * COUT], start=st, stop=sp)
            ckt = cpool.tile([128, COUT], F32R, tag="ckt")
            q12t = cpool.tile([128, 2 * COUT], F32R, tag="q12t")
            nc.vector.tensor_copy(ckt[:], ckq1_ps[:, 0:COUT])
            nc.vector.tensor_copy(q12t[:, 0:COUT], ckq1_ps[:, COUT:2 * COUT])
            nc.vector.tensor_copy(q12t[:, COUT:2 * COUT], q2_ps[:])

            a12_ps = aps.tile([128, 2, 512], F32, tag="a12_ps")
            for j in range(2):
                nc.tensor.matmul(a12_ps[:, j, :], ckt[:, j * 128:(j + 1) * 128], q12t[:],
                                 start=True, stop=True)
            psl = slice(p * COUT, (p + 1) * COUT)
            nc.vector.tensor_tensor(out=a1t[:, :, psl], in0=a1t[:, :, psl],
                                    in1=a12_ps[:, :, 0:COUT], op=mybir.AluOpType.add)
            nc.vector.tensor_tensor(out=a2t[:, :, psl], in0=a2t[:, :, psl],
                                    in1=a12_ps[:, :, COUT:2 * COUT], op=mybir.AluOpType.add)


def _phase_c(nc, tc, ctx, x1, attn, u_out):
    """U = conv(x1, attn) in natural [cout, pixel] layout."""
    cbpool = ctx.enter_context(tc.tile_pool(name="cbands", bufs=1))
    spool = ctx.enter_context(tc.tile_pool(name="stage", bufs=2))
    cps = ctx.enter_context(tc.tile_pool(name="cps", bufs=2, space="PSUM"))

    CBR = 18
    cb = [cbpool.tile([128, 2, CBR, 98], F32R, tag=f"cb{i}", name=f"cb{i}") for i in range(2)]
    for bt in cb:
        nc.gpsimd.memset(bt[:].bitcast(F32), 0.0)

    for g in range(6):
        band = cb[g % 2]
        y0 = 16 * g - 1
        r0 = max(0, -y0)
        r1 = min(CBR, 96 - y0)
        if g == 5:
            nc.gpsimd.memset(band[:, :, CBR - 1, :].bitcast(F32), 0.0)
        for j in range(2):
            nc.sync.dma_start(band[:, j, r0:r1, 1:97],
                              x1[j * 128:(j + 1) * 128, y0 + r0:y0 + r1, :])
        for i in range(2):
            ps = cps.tile([128, 4, 512], F32, tag="cps")
            idx = 0
            for t in range(K2):
                ty, tx = t // 3, t % 3
                for j in range(2):
                    lhsT = attn[:, j, t * COUT + i * 128: t * COUT + (i + 1) * 128]
                    st = (idx == 0)
                    sp = (idx == 2 * K2 - 1)
                    for b4 in range(4):
                        rr = 4 * b4 + ty
                        rhs = band[:, j, rr:rr + 4, tx:tx + 96]
                        nc.tensor.matmul(ps[:, b4, 0:384], lhsT, rhs, start=st, stop=sp)
                    idx += 1
            stage = spool.tile([128, 1536], F32, tag="stage")
            nc.vector.tensor_copy(stage[:], ps[:, :, 0:384])
            nc.sync.dma_start(u_out[i * 128:(i + 1) * 128, 16 * g * 96:(16 * g + 16) * 96],
                              stage[:])


def _emit(nc, tc, x1, x2, wkq1, wq2, pos1t, pos2t, u_out, stats_out):
    with tc.tile_pool(name="accum", bufs=1) as apool:
        a1t = apool.tile([128, 2, K2 * COUT], F32, tag="a1t")
        a2t = apool.tile([128, 2, K2 * COUT], F32, tag="a2t")

        with ExitStack() as ctx:
            _phase_b(nc, tc, ctx, x1, x2, wkq1, wq2, a1t, a2t, pos1t, pos2t)

        with ExitStack() as ctx2:
            atpool = ctx2.enter_context(tc.tile_pool(name="attn", bufs=1))
            spool2 = ctx2.enter_context(tc.tile_pool(name="stats", bufs=1))

            attn = atpool.tile([128, 2, K2 * COUT], F32R, tag="attn")
            nc.vector.tensor_tensor(out=attn[:], in0=a1t[:], in1=a2t[:], op=mybir.AluOpType.mult)

            attn_f = attn[:].bitcast(F32)
            stat = spool2.tile([128, 2], F32, tag="stat")
            nc.vector.reduce_sum(stat[:, 0:1], attn_f, axis=mybir.AxisListType.XY)
            sq = spool2.tile([128, 2, K2 * COUT], F32, tag="sq")
            nc.scalar.activation(sq[:], attn_f, mybir.ActivationFunctionType.Square,
                                 accum_out=stat[:, 1:2])
            nc.sync.dma_start(stats_out[:], stat[:])

            with ExitStack() as ctx3:
                _phase_c(nc, tc, ctx3, x1, attn, u_out)

def _prep_w(w):
    # [Cout, Cin, 3, 3] -> [cin, (ty*3+tx)*256 + cout]
    return np.ascontiguousarray(
        w.transpose(1, 2, 3, 0).reshape(CIN, K2 * COUT), dtype=np.float32)


def _prep_pos(pos):
    # [Cout, Cin*9] -> [cin, p*256 + cout]
    return np.ascontiguousarray(
        pos.reshape(COUT, CIN, K2).transpose(1, 2, 0).reshape(CIN, K2 * COUT),
        dtype=np.float32)


def build_in_maps(x1, x2, key_w, q1_w, q2_w, pos1, pos2):
    wk_r, wq1_r, wq2_r = _prep_w(key_w), _prep_w(q1_w), _prep_w(q2_w)
    wkq1_r = np.ascontiguousarray(
        np.concatenate([wk_r.reshape(CIN, K2, COUT), wq1_r.reshape(CIN, K2, COUT)],
                       axis=2).reshape(CIN, K2 * 2 * COUT))
    p1_r, p2_r = _prep_pos(pos1), _prep_pos(pos2)
    return [{
        "x1": np.ascontiguousarray(x1[b]),
        "x2": np.ascontiguousarray(x2[b]),
        "wkq1": wkq1_r, "wq2": wq2_r,
        "pos1t": p1_r, "pos2t": p2_r,
    } for b in range(N_CORES)]


def kernel(**inputs):
    x1 = np.asarray(inputs["x1"], dtype=np.float32)
    x2 = np.asarray(inputs["x2"], dtype=np.float32)
    key_w = np.asarray(inputs["key_w"], dtype=np.float32)
    q1_w = np.asarray(inputs["q1_w"], dtype=np.float32)
    q2_w = np.asarray(inputs["q2_w"], dtype=np.float32)
    pos1 = np.asarray(inputs["pos1"], dtype=np.float32)
    pos2 = np.asarray(inputs["pos2"], dtype=np.float32)
    momentum = float(np.asarray(inputs["kernel_momentum"]))

    assert x1.shape == (B, CIN, H, W), x1.shape

    if "nc" not in _CACHE:
        _CACHE["nc"] = build_nc()
    nc = _CACHE["nc"]

    in_maps = build_in_maps(x1, x2, key_w, q1_w, q2_w, pos1, pos2)
    _CACHE["last_in_maps"] = in_maps
    res = run_bass_kernel_spmd(nc, in_maps, list(range(N_CORES)))

    U = np.stack([res.results[b]["u"] for b in range(N_CORES)])          # [B, Cout, HW]
    stats = np.stack([res.results[b]["stats"] for b in range(N_CORES)])  # [B, 128, 2]

    s1 = float(stats[:, :, 0].astype(np.float64).sum())
    s2 = float(stats[:, :, 1].astype(np.float64).sum())
    n = float(B * COUT * CIN * K2)
    mu = s1 / n
    var = (s2 - s1 * s1 / n) / (n - 1.0)
    sd = np.sqrt(max(var, 0.0))
    scale = NORM_SCALE / (sd + 1e-4)
    a = momentum + scale
    bb = -mu * scale

    # boxsum(x1): conv(x1, all-ones weights) — identical for every out channel
    t = x1.sum(axis=1)                                   # [B, 96, 96]
    tp = np.pad(t, ((0, 0), (1, 1), (1, 1)))
    s = np.zeros_like(t)
    for ty in range(3):
        for tx in range(3):
            s += tp[:, ty:ty + 96, tx:tx + 96]

    out = a * U.reshape(B, COUT, H, W) + bb * s[:, None, :, :]
    return out.astype(np.float32)
